# revision 1
# baseline (speedup 1.0000x reference)
"""Causal single-head attention (B=2, T=4096, C=1024, D=64) on 8 TRN2 cores.

Sharding: core i -> batch b = i//4, query chunk c = i%4 (rows Q0=1024c..Q0+1024).
One SPMD Bass program; per-core causal structure is entirely data-driven:
  - x[b] is rolled on host so the core's own query rows occupy key slots
    3072..4095 ("band"); the diagonal tril there is a compile-time
    affine_select, identical on every core.
  - every other key slot is fully-visible or fully-masked per core, encoded
    in a per-key additive bias folded in as a 65th contraction row of the
    S^T = K'.T@Q' matmul (row 64 of K'^T = bias, row 64 of Q'^T = 1).
  - denominator comes for free from a ones-column in V' (column 64), so the
    kernel returns unnormalized [65, 1024] = [PV^T ; rowsum]; host divides.
x is sent pre-transposed [C, T] so the kernel needs no on-device transposes
except a cheap [64,128] PE transpose of V^T -> V.
"""

import numpy as np

B, T, C, D = 2, 4096, 1024, 64
NCORES = 8
TQ = 1024          # queries per core
NKT = T // 128     # 32 key tiles of 128
BAND_KT0 = 24      # band = key tiles 24..31 (slots 3072..4095)
NEG = -1e30
DTYPE_NAME = "bfloat16"  # compute/storage dtype for x, weights, K/V/Q, P

_CACHE = {}


def _dtypes():
    import concourse.mybir as mybir
    if DTYPE_NAME == "bfloat16":
        import ml_dtypes
        return mybir.dt.bfloat16, ml_dtypes.bfloat16
    return mybir.dt.float32, np.float32


def _build_program(dt_x):
    import concourse.bass as bass
    import concourse.mybir as mybir
    import concourse.tile as tile
    from concourse import bacc
    from concourse.masks import make_identity
    from contextlib import ExitStack

    f32 = mybir.dt.float32

    nc = bacc.Bacc(
        "TRN2",
        target_bir_lowering=False,
        debug=False,
        num_devices=NCORES,
    )

    xT_t = nc.dram_tensor("xT", [C, T], dt_x, kind="ExternalInput")
    kb_t = nc.dram_tensor("kb", [1, T], dt_x, kind="ExternalInput")
    wkv_t = nc.dram_tensor("wkv", [128, 8, 128], dt_x, kind="ExternalInput")
    wq_t = nc.dram_tensor("wq", [128, 8, 64], dt_x, kind="ExternalInput")
    out_t = nc.dram_tensor("outT", [65, TQ], f32, kind="ExternalOutput")

    xT = xT_t.ap()
    kb = kb_t.ap()
    wkv = wkv_t.ap()
    wq = wq_t.ap()
    outT = out_t.ap()

    with tile.TileContext(nc) as tc, ExitStack() as ctx:
        const = ctx.enter_context(tc.tile_pool(name="const", bufs=1))
        xpool = ctx.enter_context(tc.tile_pool(name="xpool", bufs=8))
        stage = ctx.enter_context(tc.tile_pool(name="stage", bufs=3))
        ppool = ctx.enter_context(tc.tile_pool(name="ppool", bufs=3))
        psA = ctx.enter_context(tc.tile_pool(name="psA", bufs=2, space="PSUM"))
        psP = ctx.enter_context(tc.tile_pool(name="psP", bufs=2, space="PSUM"))
        psO = ctx.enter_context(tc.tile_pool(name="psO", bufs=1, space="PSUM"))

        # persistent SBUF tensors
        KT = const.tile([65, T], dt_x)        # K'^T: rows 0..63 = K^T, row 64 = key bias
        VS = const.tile([128, NKT, 65], dt_x)  # V': [:, kt, 0:64] = V rows, col 64 = 1
        QT = const.tile([65, TQ], dt_x)       # Q'^T: row 64 = 1
        wkv_sb = const.tile([128, 8, 128], dt_x)
        wq_sb = const.tile([128, 8, 64], dt_x)
        ident = const.tile([64, 64], dt_x)

        # two HWDGE queues: SP carries chunk 6 + even chunks + output,
        # Activation carries weights/bias/masks + chunk 7 + odd chunks
        nc.scalar.dma_start(out=wkv_sb, in_=wkv)
        nc.scalar.dma_start(out=wq_sb, in_=wq)
        nc.scalar.dma_start(out=KT[64:65, :], in_=kb)
        nc.vector.memset(QT[64:65, :], 1.0)
        nc.vector.memset(VS[:, :, 64:65], 1.0)
        make_identity(nc, ident)

        # band tril masks (1 where q_local >= k_local, else 0), built once
        # on otherwise-idle DVE/Pool during the DMA-bound start; applied
        # post-exp as a DVE multiply
        trilm = const.tile([128, 8, TQ], dt_x)
        nc.vector.memset(trilm, 1.0)
        for bk in range(8):
            nc.gpsimd.affine_select(
                out=trilm[:, bk, :],
                in_=trilm[:, bk, :],
                compare_op=mybir.AluOpType.is_ge,
                fill=0.0,
                base=-128 * bk,
                pattern=[[1, TQ]],
                channel_multiplier=-1,
            )

        xT_r = xT.rearrange("(a p) t -> p a t", p=128)  # [128, 8, T]

        # prefetch all 8 chunks up front (query chunks 6,7 first, per-cc
        # granularity so the first matmuls start after 1/8 of a chunk)
        xts = {}
        for tci in (6, 7, 0, 1, 2, 3, 4, 5):
            ts = slice(tci * 512, (tci + 1) * 512)
            xt = xpool.tile([128, 8, 512], dt_x, tag="xt")
            if tci >= 6:
                for cc in range(8):
                    nc.sync.dma_start(out=xt[:, cc, :], in_=xT_r[:, cc, ts])
            else:
                nc.sync.dma_start(out=xt, in_=xT_r[:, :, ts])
            xts[tci] = xt

        vts = {}

        def proj_mm(tci):
            """KV (+Q) projection matmuls for chunk tci -> KT cols, vt stage."""
            ts = slice(tci * 512, (tci + 1) * 512)
            xt = xts[tci]

            kv_ps = psP.tile([128, 512], f32, tag="pj")
            for cc in range(8):
                nc.tensor.matmul(
                    kv_ps,
                    lhsT=wkv_sb[:, cc, :],
                    rhs=xt[:, cc, :],
                    start=(cc == 0),
                    stop=(cc == 7),
                )
            if tci >= 6:
                q_ps = psP.tile([64, 512], f32, tag="pj")
                for cc in range(8):
                    nc.tensor.matmul(
                        q_ps,
                        lhsT=wq_sb[:, cc, :],
                        rhs=xt[:, cc, :],
                        start=(cc == 0),
                        stop=(cc == 7),
                    )
                qs = slice((tci - 6) * 512, (tci - 5) * 512)
                nc.vector.tensor_copy(QT[0:64, qs], q_ps)
            nc.vector.tensor_copy(KT[0:64, ts], kv_ps[0:64, :])
            vt = stage.tile([64, 512], dt_x, tag="vt")
            nc.vector.tensor_copy(vt, kv_ps[64:128, :])
            vts[tci] = vt

        def v_fixup(tci):
            """Transpose V^T chunk -> VS tiles (deferred off critical path)."""
            vt = vts.pop(tci)
            # pack 4 transposes into one PSUM bank (first sets start=True to
            # clear has_written bits; rest overwrite their own regions)
            vq = psP.tile([128, 4, 64], dt_x, tag="pj")
            for sub in range(4):
                nc.tensor.matmul(
                    vq[:, sub, :],
                    lhsT=vt[:, sub * 128:(sub + 1) * 128],
                    rhs=ident,
                    is_transpose=True,
                    start=(sub == 0),
                    stop=(sub == 3),
                    skip_group_check=True,
                )
            nc.vector.tensor_copy(VS[:, tci * 4:tci * 4 + 4, 0:64], vq)

        def proj_pieces(tci):
            """proj_chunk as a list of small closures, to smear across the
            attend pipeline so PE load stays even."""
            ts = slice(tci * 512, (tci + 1) * 512)
            xt = xts[tci]
            kv_ps = psP.tile([128, 512], f32, tag="pj")

            def mk(cc0):
                def f():
                    for cc in (cc0, cc0 + 1):
                        nc.tensor.matmul(
                            kv_ps,
                            lhsT=wkv_sb[:, cc, :],
                            rhs=xt[:, cc, :],
                            start=(cc == 0),
                            stop=(cc == 7),
                        )
                return f

            def finish():
                qs = slice(tci * 512, (tci + 1) * 512)
                nc.vector.tensor_copy(KT[0:64, qs], kv_ps[0:64, :])
                vt = stage.tile([64, 512], dt_x, tag="vt")
                nc.vector.tensor_copy(vt, kv_ps[64:128, :])
                vts[tci] = vt

            return [mk(0), mk(2), mk(4), mk(6), finish,
                    lambda: v_fixup(tci)]

        pv = psO.tile([65, TQ], f32)

        def attend_S(kt):
            """S^T matmuls for one key tile (PE)."""
            s_ps = psA.tile([128, TQ], f32, tag="s")
            for qh in range(2):
                qs = slice(qh * 512, (qh + 1) * 512)
                nc.tensor.matmul(
                    s_ps[:, qs],
                    lhsT=KT[:, kt * 128:(kt + 1) * 128],
                    rhs=QT[:, qs],
                    start=True,
                    stop=True,
                )
            return s_ps

        def attend_rest(kt, s_ps, first, last):
            """exp -> (band tril) -> PV accumulate for one key tile."""
            p_sb = ppool.tile([128, TQ], dt_x, tag="p")
            nc.scalar.activation(
                p_sb, s_ps, mybir.ActivationFunctionType.Exp, scale=float(D) ** -0.5
            )
            if kt >= BAND_KT0:
                # band tril: zero P above the diagonal (prebuilt 0/1 mask)
                nc.vector.tensor_mul(p_sb, p_sb, trilm[:, kt - BAND_KT0, :])
            for qh in range(2):
                qs = slice(qh * 512, (qh + 1) * 512)
                nc.tensor.matmul(
                    pv[:, qs],
                    lhsT=VS[:, kt, :],
                    rhs=p_sb[:, qs],
                    start=first,
                    stop=last,
                )

        # Software-pipelined, interleaved schedule: query chunks (6,7)
        # projected first so band key-tiles can attend immediately; the
        # S^T matmul of kt+1 is emitted before PV of kt so PE never stalls
        # on ACT's exp; remaining projections fill PE gaps.
        proj_mm(6)
        proj_mm(7)
        v_fixup(6)
        v_fixup(7)
        order = list(range(BAND_KT0, NKT)) + list(range(BAND_KT0))

        pending = []   # [(tci, closure)] proj pieces smeared across attends
        queued = set()

        def queue_chunk(c):
            if c in queued or not (0 <= c <= 5):
                return
            queued.add(c)
            pending.extend((c, f) for f in proj_pieces(c))

        def drain_chunk(c):
            rest = []
            for tc, f in pending:
                if tc == c:
                    f()
                else:
                    rest.append((tc, f))
            pending[:] = rest

        queue_chunk(0)
        queue_chunk(1)
        pipe = []  # [(kt, s_ps)]
        for kt in order:
            if kt < BAND_KT0 and kt % 4 == 0:
                drain_chunk(kt // 4)
                queue_chunk(kt // 4 + 2)
            pipe.append((kt, attend_S(kt)))
            if len(pipe) > 1:
                pkt, ps = pipe.pop(0)
                attend_rest(pkt, ps, first=(pkt == order[0]), last=False)
            for _ in range(2):
                if pending:
                    tc, f = pending.pop(0)
                    f()
        pkt, ps = pipe.pop(0)
        attend_rest(pkt, ps, first=False, last=True)

        osb = stage.tile([65, TQ], f32, tag="o")
        for qh in range(2):  # halves so copy/DMA overlap the last PV matmul
            qs = slice(qh * 512, (qh + 1) * 512)
            nc.vector.tensor_copy(osb[:, qs], pv[:, qs])
            nc.sync.dma_start(out=outT[:, qs], in_=osb[:, qs])

    nc.compile()
    return nc


def _prep_inputs(x, Wq, Wk, Wv, np_dt):
    """Per-core input maps."""
    wkv = np.empty((128, 8, 128), dtype=np_dt)
    wkv[:, :, 0:64] = Wk.reshape(8, 128, 64).transpose(1, 0, 2)
    wkv[:, :, 64:128] = Wv.reshape(8, 128, 64).transpose(1, 0, 2)
    wq = np.ascontiguousarray(
        Wq.reshape(8, 128, 64).transpose(1, 0, 2)).astype(np_dt)

    # band tril masks: trilm[p, bk, q] = 1.0 if q >= 128*bk + p else 0.0
    q_idx = np.arange(TQ)
    trilm = np.empty((128, 8, TQ), dtype=np_dt)
    for bk in range(8):
        for p_row in range(128):
            trilm[p_row, bk, :] = (q_idx >= 128 * bk + p_row)

    in_maps = []
    for core in range(NCORES):
        b, c = divmod(core, 4)
        Q0 = TQ * c
        xr = np.roll(x[b], -(Q0 + TQ), axis=0)  # slot s -> abs key (s+Q0+TQ)%T
        xT = np.ascontiguousarray(xr.T).astype(np_dt)
        absk = (np.arange(T) + Q0 + TQ) % T
        kbias = np.where(absk < Q0, 0.0, NEG).astype(np_dt)
        kbias[T - TQ:] = 0.0  # band slots: tril handled on-device
        in_maps.append({
            "xT": xT,
            "kb": kbias.reshape(1, T),
            "wkv": wkv,
            "wq": wq,
        })
    return in_maps


def kernel(x, Wq, Wk, Wv, _trace=False):
    from concourse.bass_utils import run_bass_kernel_spmd

    dt_x, np_dt = _dtypes()

    key = ("prog", str(dt_x))
    if key not in _CACHE:
        _CACHE[key] = _build_program(dt_x)
    nc = _CACHE[key]

    in_maps = _prep_inputs(
        np.asarray(x, np.float32), np.asarray(Wq, np.float32),
        np.asarray(Wk, np.float32), np.asarray(Wv, np.float32), np_dt)

    res = run_bass_kernel_spmd(
        nc, in_maps, core_ids=list(range(NCORES)), trace=_trace)

    out = np.empty((B, T, D), dtype=np.float32)
    for core in range(NCORES):
        b, c = divmod(core, 4)
        o = res.results[core]["outT"]  # [65, TQ]
        out[b, TQ * c:TQ * (c + 1), :] = (o[0:64, :] / o[64:65, :]).T
    if _trace:
        return out, res
    return out



# revision 2
# speedup vs baseline: 29.5909x; 29.5909x over previous
"""Causal single-head attention (B=2, T=4096, C=1024, D=64) on 8 TRN2 cores.

Sharding: core i -> batch b = i//4, query chunk c = i%4 (1024 queries each).
Each core gets its batch's full x, ROLLED on host so its own query rows land
in key slots 3072..4095 ("band"); causality is then data-driven and identical
on every core:
  - key slots 0..3071 are fully-visible or fully-masked per core, encoded in
    a per-key 0/1 vector kb (multiplies exp(aff) as a rank-1 broadcast);
  - the band gets a static [1024,1024] tril mask, same on every core.
The denominator comes free from a ones-column appended to V (row-sum runs in
f32 on the PE during the PV matmul); a final f32 divide normalizes.

Math runs in bf16 (inputs pre-cast on host, scale folded into Wq; scores are
exp'd in bf16 straight out of the QK^T matmul). Executed as a plain XLA
program under shard_map (compiled by neuronx-cc), which has far lower
per-execute overhead than the bass_exec custom-call path on this stack.
"""

import numpy as np

B, T, C, D = 2, 4096, 1024, 64
NCORES = 8
TQ = 1024          # queries per core
NV = T - TQ        # non-band key slots (3072)

_CACHE = {}


def _build():
    import jax
    import jax.numpy as jnp
    from jax.sharding import Mesh, PartitionSpec, NamedSharding
    from jax.experimental.shard_map import shard_map

    devices = jax.devices()[:NCORES]
    mesh = Mesh(np.asarray(devices), ("core",))
    shard = NamedSharding(mesh, PartitionSpec("core"))
    repl = NamedSharding(mesh, PartitionSpec())
    bf = jnp.bfloat16

    def attend(xr, kb, tril, Wq, Wkv):
        # xr [T, C] rolled keys (queries = last TQ rows); kb [1, NV] 0/1;
        # tril [TQ, TQ]; Wq [C, D] (pre-scaled); Wkv [C, 2D]
        q = xr[NV:] @ Wq                  # [TQ, D]
        kv = xr @ Wkv                     # [T, 2D]
        k = kv[:, :D]
        v = kv[:, D:]
        ve = jnp.concatenate([v, jnp.ones((T, 1), bf)], axis=1)   # [T, D+1]
        aff = q @ k.T                     # [TQ, T]
        m = jnp.concatenate([jnp.broadcast_to(kb, (TQ, NV)), tril], axis=1)
        p = jnp.exp(aff) * m              # [TQ, T] bf16
        oe = (p @ ve).astype(jnp.float32)  # [TQ, D+1]
        return oe[:, :D] / oe[:, D:]      # [TQ, D] f32

    def body(xr, kb, tril, Wq, Wkv):
        return attend(xr, kb, tril, Wq, Wkv)

    sharded = jax.jit(shard_map(
        body, mesh=mesh,
        in_specs=(PartitionSpec("core"), PartitionSpec("core"),
                  PartitionSpec(), PartitionSpec(), PartitionSpec()),
        out_specs=PartitionSpec("core"), check_rep=False))
    return sharded, attend, mesh, shard, repl


def _get_built():
    if "built" not in _CACHE:
        _CACHE["built"] = _build()
    return _CACHE["built"]


def _prep_inputs(x, Wq, Wk, Wv):
    """Host-side prep: per-core rolled bf16 x, kb vectors, weights."""
    import ml_dtypes
    nbf = ml_dtypes.bfloat16

    scale = np.float32(D ** -0.5)
    Wq_s = (np.asarray(Wq, np.float32) * scale).astype(nbf)
    Wkv = np.concatenate(
        [np.asarray(Wk, np.float32), np.asarray(Wv, np.float32)],
        axis=1).astype(nbf)

    xr = np.empty((NCORES, T, C), dtype=nbf)
    kb = np.zeros((NCORES, 1, NV), dtype=nbf)
    x = np.asarray(x, np.float32)
    for i in range(NCORES):
        b, c = divmod(i, 4)
        Q0 = TQ * c
        # abs key a -> slot (a - Q0 - TQ) mod T; own queries at slots NV..T-1
        xr[i] = np.roll(x[b], -(Q0 + TQ), axis=0)
        if Q0:
            kb[i, 0, NV - Q0:] = 1  # slots NV-Q0..NV-1 hold visible keys 0..Q0-1
    tril = np.tril(np.ones((TQ, TQ))).astype(nbf)
    return xr.reshape(NCORES * T, C), kb.reshape(NCORES, NV), tril, Wq_s, Wkv


def kernel(x, Wq, Wk, Wv):
    import jax

    sharded, _attend, mesh, shard, repl = _get_built()
    xr, kb, tril, Wq_s, Wkv = _prep_inputs(x, Wq, Wk, Wv)

    args = [jax.device_put(xr, shard), jax.device_put(kb, shard),
            jax.device_put(tril, repl), jax.device_put(Wq_s, repl),
            jax.device_put(Wkv, repl)]
    r = sharded(*args)
    out8 = np.asarray(r).reshape(NCORES, TQ, D).astype(np.float32)

    out = np.empty((B, T, D), dtype=np.float32)
    for i in range(NCORES):
        b, c = divmod(i, 4)
        out[b, TQ * c:TQ * (c + 1), :] = out8[i]
    return out


# revision 3
# speedup vs baseline: 36.7169x; 1.2408x over previous
"""Causal single-head attention (B=2, T=4096, C=1024, D=64) on 8 TRN2 cores.

Sharding: core i -> batch b = i//4, query chunk c = i%4 (1024 queries each).
Each core gets its batch's full x, ROLLED on host so its own query rows land
in key slots 3072..4095 ("band"); causality is then data-driven and identical
on every core:
  - key slots 0..3071 are fully-visible or fully-masked per core, encoded in
    a per-key 0/1 column kbT (multiplies exp(affT) as a rank-1 broadcast);
  - the band gets a static [1024,1024] tril^T mask, same on every core.

The whole computation is written transposed / contraction-major so every dot
maps onto the PE without large layout transposes (mirrors the hand-written
Bass kernel's S^T/P^T/out^T scheme):
  x arrives as xrT [C, T]; kvT = Wkv^T xrT ([2D, T]); qT = Wq^T xrT[:,band];
  affT = kvT[:D]^T-free einsum -> [T, TQ]; pT = exp(affT) * mT;
  oeT = ve^T-contracted einsum -> [D+1, TQ].
The only explicit transpose is vT -> v ([64, 4096] bf16, 0.5MB). The
denominator comes free from a ones-column appended to V (row-sum accumulates
in f32 on the PE during the PV matmul); a final f32 divide normalizes.

Math runs in bf16 (inputs pre-cast on host, scale folded into Wq). Executed
as a plain XLA program under shard_map (compiled by neuronx-cc), which has
far lower per-execute overhead than the bass_exec custom-call path on this
stack.
"""

import numpy as np

B, T, C, D = 2, 4096, 1024, 64
NCORES = 8
TQ = 1024          # queries per core
NV = T - TQ        # non-band key slots (3072)

_CACHE = {}


def _build():
    import jax
    import jax.numpy as jnp
    from jax.sharding import Mesh, PartitionSpec, NamedSharding
    from jax.experimental.shard_map import shard_map

    devices = jax.devices()[:NCORES]
    mesh = Mesh(np.asarray(devices), ("core",))
    shard = NamedSharding(mesh, PartitionSpec("core"))
    repl = NamedSharding(mesh, PartitionSpec())
    bf = jnp.bfloat16

    def attend(xrT, kbT, trilT, Wq, Wkv):
        # xrT [C, T] rolled keys (queries = last TQ cols); kbT [NV, 1] 0/1;
        # trilT [TQ, TQ]; Wq [C, D] (pre-scaled); Wkv [C, 2D]
        kvT = jnp.einsum('cd,ct->dt', Wkv, xrT)        # [2D, T]
        kT = kvT[:D]                                   # [D, T]
        v = kvT[D:].T                                  # [T, D]
        qT = jnp.einsum('cd,ct->dt', Wq, xrT[:, NV:])  # [D, TQ]
        ve = jnp.concatenate([v, jnp.ones((T, 1), bf)], axis=1)   # [T, D+1]
        affT = jnp.einsum('dk,dq->kq', kT, qT)         # [T, TQ]
        mT = jnp.concatenate([jnp.broadcast_to(kbT, (NV, TQ)), trilT], axis=0)
        pT = jnp.exp(affT) * mT                        # [T, TQ] bf16
        oeT = jnp.einsum('ke,kq->eq', ve, pT).astype(jnp.float32)  # [D+1, TQ]
        return oeT[:D] / oeT[D:]                       # [D, TQ] f32

    def body(xrT, kbT, trilT, Wq, Wkv):
        return attend(xrT, kbT, trilT, Wq, Wkv)

    sharded = jax.jit(shard_map(
        body, mesh=mesh,
        in_specs=(PartitionSpec("core"), PartitionSpec("core"),
                  PartitionSpec(), PartitionSpec(), PartitionSpec()),
        out_specs=PartitionSpec("core"), check_rep=False))
    return sharded, attend, mesh, shard, repl


def _get_built():
    if "built" not in _CACHE:
        _CACHE["built"] = _build()
    return _CACHE["built"]


def _prep_inputs(x, Wq, Wk, Wv):
    """Host-side prep: per-core rolled+transposed bf16 x, kbT columns, weights."""
    import ml_dtypes
    nbf = ml_dtypes.bfloat16

    scale = np.float32(D ** -0.5)
    Wq_s = (np.asarray(Wq, np.float32) * scale).astype(nbf)
    Wkv = np.concatenate(
        [np.asarray(Wk, np.float32), np.asarray(Wv, np.float32)],
        axis=1).astype(nbf)

    xrT = np.empty((NCORES, C, T), dtype=nbf)
    kbT = np.zeros((NCORES, NV, 1), dtype=nbf)
    x = np.asarray(x, np.float32)
    for i in range(NCORES):
        b, c = divmod(i, 4)
        Q0 = TQ * c
        # abs key a -> slot (a - Q0 - TQ) mod T; own queries at slots NV..T-1
        xrT[i] = np.roll(x[b], -(Q0 + TQ), axis=0).T
        if Q0:
            kbT[i, NV - Q0:, 0] = 1  # slots NV-Q0..NV-1 = visible keys 0..Q0-1
    trilT = np.ascontiguousarray(np.tril(np.ones((TQ, TQ))).T).astype(nbf)
    return (xrT.reshape(NCORES * C, T), kbT.reshape(NCORES * NV, 1),
            trilT, Wq_s, Wkv)


def kernel(x, Wq, Wk, Wv):
    import jax

    sharded, _attend, mesh, shard, repl = _get_built()
    xrT, kbT, trilT, Wq_s, Wkv = _prep_inputs(x, Wq, Wk, Wv)

    args = [jax.device_put(xrT, shard), jax.device_put(kbT, shard),
            jax.device_put(trilT, repl), jax.device_put(Wq_s, repl),
            jax.device_put(Wkv, repl)]
    r = sharded(*args)
    out8 = np.asarray(r).reshape(NCORES, D, TQ).astype(np.float32)

    out = np.empty((B, T, D), dtype=np.float32)
    for i in range(NCORES):
        b, c = divmod(i, 4)
        out[b, TQ * c:TQ * (c + 1), :] = out8[i].T
    return out
